# revision 3
# baseline (speedup 1.0000x reference)
"""Eq2to2 equivariant layer (Maron et al. 2-to-2 basis, 15 ops) as a Trainium2
Bass/Tile kernel, data-parallel over the batch axis N across 8 NeuronCores.

Math: the 15-basis contraction collapses to
  out[n,s] = sum_d C9[d,s]*x[n,d] + sum_d C10[d,s]*x[n,d]^T
           + Row[n,s,i] (bcast over j) + Col[n,s,j] (bcast over i)
           + delta_ij * DiagT[n,s,i] + Const[n,s] + bias[s] + delta_ij*diag_bias[s]
where Row/Col/DiagT/Const are small contractions of rowsum/colsum/diag/tot stats.

v4, shaped by measured per-pattern engine rates on the target backend:
strided moving operands on the PE cost ~5x contiguous, one-hot (stride 0/-1)
moving operands are nearly free, and DMA sustains full rate with >=4KB
descriptors. Consequences:
- The host ships BOTH x and x^T as fp8-E3M4 (quantize once, transpose the
  fp8 array -- no extra rounding): all four per-chunk matmuls use contiguous
  or one-hot moving operands. In: 4 MiB, out: 4 MiB bf16.
- Per 512-chunk the PE runs FOUR accumulating matmuls: W9^T@x, W10^T@xT
  (both contiguous fp8), row-broadcast (rowft x anti-diag one-hot), and
  col-broadcast (colft x the same tile read with stride -1 = identity).
  colft bakes in Const+bias via stride-0 stationary aux matmuls.
- Stats avoid strided reads entirely: colsum(x) and rowsum(x)=colsum(x^T)
  both use contiguous fold trees over [P,4096] slices, split DVE/Pool.
- ACT drains psum->SBUF bf16; DVE adds only the sparse diagonal term.
- One dma_start per tensor per rep (16KB descriptors), issued one rep AHEAD
  (prefetch) so stats(k) overlap mains/drains(k-1); two 2MiB output stores.
Weights [128,16,128] bf16 (15 ops + bias row) stay resident.

Layout: each core takes 4 n's -> 128 SBUF partitions = (nq, d). Grids are flat
in the free dim (16384 elems per partition).
"""

import sys

import numpy as np

if "/opt/trn_rl_repo" not in sys.path:
    sys.path.insert(0, "/opt/trn_rl_repo")

N, D, S, B, M = 32, 32, 32, 15, 128
NCORES = 8
NPC = N // NCORES          # n's per core = 4
P = 128                    # partitions
FREE = M * M               # 16384
CHUNK = 512                # psum bank (f32)
PAIR = 1024                # psum tile width
NSLICE = 4                 # stats tree slices per tensor
SL = FREE // NSLICE        # 4096 elements (32 i-rows) per tree slice
IPS = SL // M              # i-rows per slice = 32
NGROUP = 2                 # out staging groups (one store DMA each)
GW = FREE // NGROUP        # 8192 elements per group (8 pairs)
POOL_L1 = {("a", 1), ("a", 2), ("t", 1)}  # slices whose L1 fold runs on Pool

_cache: dict = {}


def _build_program(repeat=1):
    import concourse.bass as bass
    import concourse.tile as tile
    from concourse import bacc, mybir

    f32 = mybir.dt.float32
    bf16 = mybir.dt.bfloat16
    fp8 = mybir.dt.float8e3
    nc = bacc.Bacc("TRN2", target_bir_lowering=False, debug=False)

    xr_d = nc.dram_tensor("xr", [P, FREE], fp8, kind="ExternalInput")
    xt_d = nc.dram_tensor("xt", [P, FREE], fp8, kind="ExternalInput")
    # pre-scaled coefs, block-diagonal: [(nq,d), 16, (nq,s)] (slot 15 = bias)
    wm_d = nc.dram_tensor("wmats", [P, 16, P], bf16, kind="ExternalInput")
    bc_d = nc.dram_tensor("bcols", [P, 2], f32, kind="ExternalInput")
    ad_d = nc.dram_tensor("adiag", [P, M], bf16, kind="ExternalInput")
    on_d = nc.dram_tensor("onesc", [P, 1], bf16, kind="ExternalInput")
    out_d = nc.dram_tensor("outr", [P, FREE], bf16, kind="ExternalOutput")

    ADD = mybir.AluOpType.add

    with tile.TileContext(nc) as tc:
        with (
            tc.tile_pool(name="big", bufs=3) as big,
            tc.tile_pool(name="cst", bufs=1) as cst,
            tc.tile_pool(name="scr", bufs=1) as scr,
            tc.tile_pool(name="aux", bufs=2) as aux,
            tc.tile_pool(name="ot", bufs=3) as otp,
            tc.tile_pool(name="pm", bufs=3, space="PSUM") as pmp,
            tc.tile_pool(name="pa", bufs=1, space="PSUM") as pap,
        ):
          # constants are loop-invariant: one tile set, loaded during rep 0
          wm = cst.tile([P, 16, P], bf16)
          bc = cst.tile([P, 2], f32)
          adg = cst.tile([P, M], bf16)
          onesc = cst.tile([P, 1], bf16)
          adg_ap = adg[:]
          mm = nc.tensor.matmul

          W = lambda idx: wm[:, idx, :]
          (W_X, W_XT, W_ROW_CS, W_ROW_RS, W_ROW_DG, W_COL_CS, W_COL_RS,
           W_COL_DG, W_DIA_DG, W_DIA_RS, W_DIA_CS, W_SD_SD, W_SD_TOT,
           W_SC_SD, W_SC_TOT, W_BIAS) = range(16)

          def aap(a, offset, dims):
              return bass.AP(tensor=a.tensor, offset=a.offset + offset,
                             ap=[list(a.ap[0])] + dims)

          def vap(t, offset, dims):
              return aap(t[:], offset, dims)

          def stt_add(out, in0, in1, eng=None):
              (eng or nc.vector).tensor_tensor(out=out, in0=in0, in1=in1,
                                               op=ADD)

          def emit_loads(_rep):
            # loads are issued one rep AHEAD of their consumers (prefetch) so
            # stats(k) overlap mains/drains(k-1) instead of waiting on DMA
            xa = big.tile([P, FREE], fp8, tag="xa", name=f"xa{_rep % 4}")
            xt = big.tile([P, FREE], fp8, tag="xt", name=f"xt{_rep % 4}")
            nc.sync.dma_start(out=xa[:], in_=xr_d[:])
            nc.scalar.dma_start(out=xt[:], in_=xt_d[:])
            return xa, xt

          def emit_stats(_rep, xa, xt):
            xa_ap = xa[:]
            xt_ap = xt[:]

            def xap(offset, dims):
                return aap(xa_ap, offset, dims)

            def tap(offset, dims):
                return aap(xt_ap, offset, dims)

            # contiguous fold-tree scratch: one set per (tree, engine-side)
            def tree_scr(pfx):
                return tuple(
                    scr.tile([P, w], bf16, name=f"{pfx}_{k}")
                    for k, w in enumerate([SL // 2, SL // 4, SL // 8,
                                           SL // 16, SL // 32, M]))

            sca_d = tree_scr("sca_d")
            sca_p = tree_scr("sca_p")
            sct_d = tree_scr("sct_d")
            sct_p = tree_scr("sct_p")
            sd_f = scr.tile([P, 1], f32)
            tot_f = scr.tile([P, 1], f32)

            # stats kept across the rep (read by aux matmuls)
            rowsum = aux.tile([P, M], bf16)
            colsum = aux.tile([P, M], bf16)
            diagx = aux.tile([P, M], bf16)
            sd = aux.tile([P, 1], bf16)
            tot = aux.tile([P, 1], bf16)
            rowft = aux.tile([P, M], bf16)   # RowF^T: [i, q]
            colft = aux.tile([P, M], bf16)   # (ColF+Const+bias)^T: [j, q]
            diaf = aux.tile([P, M], bf16)    # DiagF + diag_bias: [q, i]

            def emit_tree(key, t, src_ap, acc):
                # fold IPS=32 rows -> 1 within slice t (5 contiguous levels),
                # accumulate into acc across slices. Only the expensive L1
                # (fp8 reads) is offloadable to Pool; the rest stays on DVE.
                pool_l1 = (key, t) in POOL_L1
                l1_eng = nc.gpsimd if pool_l1 else nc.vector
                c1, c2, c3, c4, c5, ct = (sca_p if key == "a" else sct_p) \
                    if pool_l1 else (sca_d if key == "a" else sct_d)
                off = t * SL
                stt_add(c1[:], src_ap(off, [[1, 16 * M]]),
                        src_ap(off + 16 * M, [[1, 16 * M]]), eng=l1_eng)
                stt_add(c2[:], vap(c1, 0, [[1, 8 * M]]),
                        vap(c1, 8 * M, [[1, 8 * M]]))
                stt_add(c3[:], vap(c2, 0, [[1, 4 * M]]),
                        vap(c2, 4 * M, [[1, 4 * M]]))
                stt_add(c4[:], vap(c3, 0, [[1, 2 * M]]),
                        vap(c3, 2 * M, [[1, 2 * M]]))
                if t == 0:
                    stt_add(acc[:], vap(c4, 0, [[1, M]]),
                            vap(c4, M, [[1, M]]))
                else:
                    stt_add(ct[:], vap(c4, 0, [[1, M]]),
                            vap(c4, M, [[1, M]]))
                    stt_add(acc[:], acc[:], ct[:])

            for t in range(NSLICE):
                # diag elements of this slice (i in [32t, 32t+32))
                nc.vector.tensor_copy(
                    out=diagx[:, t * IPS:(t + 1) * IPS],
                    in_=xap((M + 1) * IPS * t, [[M + 1, IPS]]))
                # colsum from xa; rowsum = colsum-tree of xT
                emit_tree("a", t, xap, colsum)
                emit_tree("t", t, tap, rowsum)

            if _rep == 0:
                nc.sync.dma_start(out=wm[:], in_=wm_d[:])
                nc.sync.dma_start(out=bc[:], in_=bc_d[:])
                nc.sync.dma_start(out=adg[:], in_=ad_d[:])
                nc.sync.dma_start(out=onesc[:], in_=on_d[:])

            # sd, tot
            nc.vector.reduce_sum(out=sd_f[:], in_=diagx[:],
                                 axis=mybir.AxisListType.X)
            nc.vector.reduce_sum(out=tot_f[:], in_=rowsum[:],
                                 axis=mybir.AxisListType.X)
            nc.vector.tensor_scalar(out=sd[:], in0=sd_f[:], scalar1=0.0,
                                    scalar2=None, op0=ADD)
            nc.vector.tensor_scalar(out=tot[:], in0=tot_f[:], scalar1=0.0,
                                    scalar2=None, op0=ADD)
            return dict(xa=xa, xt=xt, xap=xap, rowsum=rowsum,
                        colsum=colsum, diagx=diagx, sd=sd, tot=tot,
                        rowft=rowft, colft=colft, diaf=diaf)

          def emit_aux(ctx):
            rowsum, colsum, diagx = (ctx["rowsum"], ctx["colsum"],
                                     ctx["diagx"])
            sd, tot = ctx["sd"], ctx["tot"]
            rowft, colft, diaf = ctx["rowft"], ctx["colft"], ctx["diaf"]
            # ---- aux contractions over d (partition dim) on the PE ----
            pr = pap.tile([P, CHUNK], f32)
            mm(pr[:, 0:M], diagx[:], W(W_ROW_DG), start=True, stop=False)
            mm(pr[:, 0:M], rowsum[:], W(W_ROW_RS), start=False, stop=False)
            mm(pr[:, 0:M], colsum[:], W(W_ROW_CS), start=False, stop=True)
            sd_b = vap(sd, 0, [[0, M]])     # sd broadcast along free (M)
            tot_b = vap(tot, 0, [[0, M]])
            ones_b = vap(onesc, 0, [[0, M]])
            mm(pr[:, M:2 * M], diagx[:], W(W_COL_DG), start=True, stop=False)
            mm(pr[:, M:2 * M], rowsum[:], W(W_COL_RS), start=False, stop=False)
            mm(pr[:, M:2 * M], colsum[:], W(W_COL_CS), start=False, stop=False)
            mm(pr[:, M:2 * M], sd_b, W(W_SC_SD), start=False, stop=False)
            mm(pr[:, M:2 * M], tot_b, W(W_SC_TOT), start=False, stop=False)
            mm(pr[:, M:2 * M], ones_b, W(W_BIAS), start=False, stop=True)
            # DiagT raw [q, i] in pr[2M:3M]; diag-const [q,1] in pr[3M:3M+1]
            mm(pr[:, 2 * M:3 * M], W(W_DIA_DG), diagx[:], start=True, stop=False)
            mm(pr[:, 2 * M:3 * M], W(W_DIA_RS), rowsum[:], start=False, stop=False)
            mm(pr[:, 2 * M:3 * M], W(W_DIA_CS), colsum[:], start=False, stop=True)
            mm(pr[:, 3 * M:3 * M + 1], W(W_SD_SD), sd[:], start=True, stop=False)
            mm(pr[:, 3 * M:3 * M + 1], W(W_SD_TOT), tot[:], start=False, stop=True)

            nc.vector.tensor_scalar(out=diaf[:], in0=pr[:, 2 * M:3 * M],
                                    scalar1=pr[:, 3 * M:3 * M + 1],
                                    scalar2=bc[:, 1:2], op0=ADD, op1=ADD)
            # psum->SBUF copies on DVE (fast psum reads; keeps ACT's queue
            # free of the aux->mains critical path)
            nc.vector.tensor_copy(out=rowft[:], in_=pr[:, 0:M])
            nc.vector.tensor_copy(out=colft[:], in_=pr[:, M:2 * M])

          def emit_mains(ctx):
            xa, xt = ctx["xa"], ctx["xt"]
            rowft, colft, diaf = ctx["rowft"], ctx["colft"], ctx["diaf"]
            # ---- main einsum + assembly: 4 matmuls per 512-chunk ----
            for g in range(NGROUP):
                ot = otp.tile([P, GW], bf16)
                for u in range(8):  # pairs within the group
                    up = g * 8 + u          # global pair index (0..15)
                    pm2 = pmp.tile([P, PAIR], f32, tag="pm")
                    for h in range(2):      # 512-chunks (psum bank limit)
                        c = up * 2 + h
                        i0 = 4 * c
                        ps = pm2[:, h * CHUNK:(h + 1) * CHUNK]
                        mm(ps, W(W_X), xa[:, c * CHUNK:(c + 1) * CHUNK],
                           start=True, stop=False)
                        mm(ps, W(W_XT), xt[:, c * CHUNK:(c + 1) * CHUNK],
                           start=False, stop=False)
                        # row term: anti-diag one-hot selects rows i0..i0+3
                        mm(ps, rowft[:],
                           bass.AP(tensor=adg_ap.tensor,
                                   offset=adg_ap.offset + (M - 1) - i0,
                                   ap=[list(adg_ap.ap[0]), [-1, 4], [0, M]]),
                           start=False, stop=False)
                        # col term: same tile read backwards = identity
                        mm(ps, colft[:],
                           bass.AP(tensor=adg_ap.tensor,
                                   offset=adg_ap.offset + (M - 1),
                                   ap=[list(adg_ap.ap[0]), [0, 4], [-1, M]]),
                           start=False, stop=True)
                    nc.scalar.copy(out=ot[:, u * PAIR:(u + 1) * PAIR],
                                   in_=pm2[:])
                # only the sparse diagonal term remains outside psum;
                # group g covers i in [64g, 64g+64): 64 diag positions
                dview = vap(ot, 64 * g, [[516, 16], [129, 4]])
                dsrc = vap(diaf, 64 * g, [[4, 16], [1, 4]])
                nc.vector.tensor_tensor(out=dview, in0=dview, in1=dsrc,
                                        op=ADD)
                nc.sync.dma_start(out=out_d[:, g * GW:(g + 1) * GW],
                                  in_=ot[:])

          # software-pipelined emission with load prefetch:
          # per iteration k: loads(k+1), stats(k), mains(k-1), aux(k)
          prev = None
          tiles = emit_loads(0)
          for _rep in range(repeat):
              nxt = emit_loads(_rep + 1) if _rep + 1 < repeat else None
              ctx = emit_stats(_rep, *tiles)
              if prev is not None:
                  emit_mains(prev)
              emit_aux(ctx)
              prev = ctx
              tiles = nxt
          emit_mains(prev)

    nc.compile()
    return nc


def _get_nc():
    if "nc" not in _cache:
        _cache["nc"] = _build_program()
    return _cache["nc"]


def _host_prep(coefs, bias, diag_bias):
    import ml_dtypes

    m = float(M)
    C = np.asarray(coefs, dtype=np.float32)

    def bd(b, scale=1.0):
        return C[:, :, b] * np.float32(scale)

    bias_v = np.asarray(bias, np.float32).reshape(S)
    # [16, D, S] pre-scaled coef blocks (slot 15 carries bias/D)
    wmats = np.stack([
        bd(9),              # W_X
        bd(10),             # W_XT
        bd(5, 1 / m),       # W_ROW_CS
        bd(6, 1 / m),       # W_ROW_RS
        bd(11),             # W_ROW_DG
        bd(7, 1 / m),       # W_COL_CS
        bd(8, 1 / m),       # W_COL_RS
        bd(12),             # W_COL_DG
        bd(0),              # W_DIA_DG
        bd(2, 1 / m),       # W_DIA_RS
        bd(3, 1 / m),       # W_DIA_CS
        bd(1, 1 / m),       # W_SD_SD
        bd(4, 1 / (m * m)),  # W_SD_TOT
        bd(13, 1 / m),      # W_SC_SD
        bd(14, 1 / (m * m)),  # W_SC_TOT
        np.broadcast_to(bias_v[None, :] / np.float32(D), (D, S)),  # W_BIAS
    ]).astype(np.float32)
    wmc = wmats.transpose(1, 0, 2).astype(ml_dtypes.bfloat16)  # [D, 16, S]
    wmats = np.zeros((P, 16, P), dtype=ml_dtypes.bfloat16)
    for nq in range(NPC):
        wmats[nq * D:(nq + 1) * D, :, nq * S:(nq + 1) * S] = wmc
    bcols = np.stack([
        np.tile(np.asarray(bias, np.float32).reshape(S), NPC),
        np.tile(np.asarray(diag_bias, np.float32).reshape(S), NPC),
    ], axis=1).astype(np.float32)
    return wmats, np.ascontiguousarray(bcols)


def _in_maps(inputs, coefs, bias, diag_bias):
    import ml_dtypes

    x = np.asarray(inputs, np.float32).astype(ml_dtypes.float8_e3m4)
    xt = np.ascontiguousarray(np.swapaxes(x, -1, -2))
    wmats, bcols = _host_prep(coefs, bias, diag_bias)
    adiag = np.zeros((P, M), dtype=ml_dtypes.bfloat16)
    for k in range(M):
        adiag[k, (M - 1) - k] = 1.0
    onesc = np.ones((P, 1), dtype=ml_dtypes.bfloat16)
    maps = []
    for i in range(NCORES):
        xr = x[i * NPC:(i + 1) * NPC].reshape(P, FREE)
        xtr = xt[i * NPC:(i + 1) * NPC].reshape(P, FREE)
        maps.append({"xr": np.ascontiguousarray(xr),
                     "xt": np.ascontiguousarray(xtr), "wmats": wmats,
                     "bcols": bcols, "adiag": adiag, "onesc": onesc})
    return maps


def run(inputs, coefs, bias, diag_bias, **spmd_kwargs):
    """Run on the 8 NeuronCores; returns (output, BassKernelResults)."""
    from concourse.bass_utils import run_bass_kernel_spmd

    nc = _get_nc()
    maps = _in_maps(inputs, coefs, bias, diag_bias)
    res = run_bass_kernel_spmd(nc, maps, list(range(NCORES)), **spmd_kwargs)
    out = np.concatenate(
        [np.asarray(r["outr"]).astype(np.float32).reshape(NPC, S, M, M)
         for r in res.results], axis=0
    )
    return np.ascontiguousarray(out), res


def kernel(inputs, coefs, bias, diag_bias):
    out, _ = run(inputs, coefs, bias, diag_bias)
    return out
